# revision 6
# baseline (speedup 1.0000x reference)
"""RBF-kernel covariance with rank-1 gate (KvvCov) on 8 Trainium2 cores.

out[b,n,m] = exp(-0.5*||x_n - x_m||^2 / exp(kernel_sigma)^2) * v[n] * v[m]

Sharding: data-parallel over B (4 batches) x 2-way row split -> 8 cores.
Each core computes a [2048, 4096] slab of one batch's NxN matrix.

Device math (per core), with s = -0.5/exp(2*sigma), A = -2*s:
  psum[n,m] = inner_q[n,m] - 0.5*normq[m]     (bf16 matmul, K=512 data rows
                                               + 3 "ones" rows carrying the
                                               split-precision -0.5*normq[m])
  rbf[n,m]  = exp(A*psum + s*normq[n])        (ACT exp, per-partition bias)
  out[n,m]  = rbf * v[m] * v[n]               (DVE tensor ops)

normq is the squared norm of the *bf16-quantized* basis vectors, so the
diagonal argument cancels to ~0 exactly as the matmul computes it.
"""

import os

import ml_dtypes
import numpy as np

import concourse.bacc as bacc
import concourse.mybir as mybir
from concourse.bass_utils import run_bass_kernel_spmd
from concourse.tile import TileContext

B, N, D = 4, 4096, 512
N_CORES = 8
ROWS = N // 2          # rows per core
R_CHUNKS = ROWS // 128  # 16 row chunks of 128
C_BLOCKS = N // 512     # 8 col blocks of 512
K_CHUNKS = D // 128     # 4 contraction chunks

BF16 = ml_dtypes.bfloat16


def _build_bass(scale_a: float):
    nc = bacc.Bacc()

    xt_d = nc.dram_tensor("xt", [128, K_CHUNKS, N], mybir.dt.bfloat16, kind="ExternalInput")
    lt_d = nc.dram_tensor("lt", [128, K_CHUNKS, ROWS], mybir.dt.bfloat16, kind="ExternalInput")
    aug_d = nc.dram_tensor("aug", [3, N], mybir.dt.bfloat16, kind="ExternalInput")
    bias_d = nc.dram_tensor("bias", [128, R_CHUNKS], mybir.dt.float32, kind="ExternalInput")
    vrows_d = nc.dram_tensor("vrows", [128, R_CHUNKS], mybir.dt.float32, kind="ExternalInput")
    vb_d = nc.dram_tensor("vb", [128, N], mybir.dt.float32, kind="ExternalInput")
    out_d = nc.dram_tensor("out", [ROWS, N], mybir.dt.float32, kind="ExternalOutput")

    with TileContext(nc) as tc:
        with (
            tc.tile_pool(name="const", bufs=1) as cpool,
            tc.tile_pool(name="exp", bufs=4) as epool,
            tc.tile_pool(name="gate", bufs=4) as gpool,
            tc.tile_pool(name="psum", bufs=6, space="PSUM") as ppool,
        ):
            xt = cpool.tile([128, K_CHUNKS, N], mybir.dt.bfloat16)
            nc.sync.dma_start(out=xt[:], in_=xt_d[:])
            lt = cpool.tile([128, K_CHUNKS, ROWS], mybir.dt.bfloat16)
            nc.sync.dma_start(out=lt[:], in_=lt_d[:])
            aug = cpool.tile([3, N], mybir.dt.bfloat16)
            nc.sync.dma_start(out=aug[:], in_=aug_d[:])
            biast = cpool.tile([128, R_CHUNKS], mybir.dt.float32)
            nc.sync.dma_start(out=biast[:], in_=bias_d[:])
            vrows = cpool.tile([128, R_CHUNKS], mybir.dt.float32)
            nc.sync.dma_start(out=vrows[:], in_=vrows_d[:])
            vb = cpool.tile([128, N], mybir.dt.float32)
            nc.sync.dma_start(out=vb[:], in_=vb_d[:])
            ones3 = cpool.tile([3, 128], mybir.dt.bfloat16)
            nc.vector.memset(ones3[:], 1.0)

            for r in range(R_CHUNKS):
                for c in range(C_BLOCKS):
                    cs = slice(c * 512, (c + 1) * 512)
                    ps = ppool.tile([128, 512], mybir.dt.float32)
                    for k in range(K_CHUNKS):
                        nc.tensor.matmul(
                            ps[:],
                            lhsT=lt[:, k, r * 128:(r + 1) * 128],
                            rhs=xt[:, k, cs],
                            start=(k == 0),
                            stop=False,
                        )
                    nc.tensor.matmul(
                        ps[:], lhsT=ones3[:], rhs=aug[:, cs], start=False, stop=True
                    )
                    e = epool.tile([128, 512], mybir.dt.float32)
                    nc.scalar.activation(
                        e[:], ps[:], mybir.ActivationFunctionType.Exp,
                        bias=biast[:, r:r + 1], scale=float(scale_a),
                    )
                    g = gpool.tile([128, 512], mybir.dt.float32)
                    nc.vector.tensor_mul(out=g[:], in0=e[:], in1=vb[:, cs])
                    nc.vector.tensor_scalar_mul(g[:], g[:], vrows[:, r:r + 1])
                    nc.sync.dma_start(
                        out=out_d[r * 128:(r + 1) * 128, cs], in_=g[:]
                    )
    nc.compile()
    return nc


def build_in_maps(emb: np.ndarray, s: float) -> list:
    """Host-side prep: per-core input tensors (slice/cast/transpose/norms)."""
    in_maps = []
    per_batch = {}
    for b in range(B):
        x = emb[b, :, :D]                       # [N, D] f32
        v = np.ascontiguousarray(emb[b, :, D])  # [N] f32
        xq = x.astype(BF16)                     # quantized basis
        xqf = xq.astype(np.float64)
        normq = (xqf * xqf).sum(axis=1)         # [N] f64, exact-ish

        # split-precision parts of -0.5*normq (3 bf16 terms)
        t = -0.5 * normq
        p0 = t.astype(BF16)
        r1 = t - p0.astype(np.float64)
        p1 = r1.astype(BF16)
        r2 = r1 - p1.astype(np.float64)
        p2 = r2.astype(BF16)
        aug = np.stack([p0, p1, p2])            # [3, N] bf16

        # xt[p, k, m] = xq[m, k*128+p]
        xt = np.ascontiguousarray(
            xq.T.reshape(K_CHUNKS, 128, N).transpose(1, 0, 2)
        )
        vb = np.ascontiguousarray(np.broadcast_to(v, (128, N)))
        per_batch[b] = (x, v, xq, normq, aug, xt, vb)

    for core in range(N_CORES):
        b, half = divmod(core, 2)
        x, v, xq, normq, aug, xt, vb = per_batch[b]
        r0 = half * ROWS
        rows = slice(r0, r0 + ROWS)
        # lt[p, k, m] = xq[r0+m, k*128+p]
        lt = np.ascontiguousarray(
            xq[rows].T.reshape(K_CHUNKS, 128, ROWS).transpose(1, 0, 2)
        )
        bias = np.ascontiguousarray(
            (s * normq[rows]).astype(np.float32).reshape(R_CHUNKS, 128).T
        )
        vrows = np.ascontiguousarray(v[rows].reshape(R_CHUNKS, 128).T)
        in_maps.append(
            {"xt": xt, "lt": lt, "aug": aug, "bias": bias, "vrows": vrows, "vb": vb}
        )
    return in_maps


def kernel(embeddings: np.ndarray, kernel_sigma: np.ndarray, num_basis_dim) -> np.ndarray:
    assert embeddings.shape == (B, N, D + 1), embeddings.shape
    nd = int(np.asarray(num_basis_dim))
    assert nd == D, nd

    sigma = float(np.asarray(kernel_sigma).reshape(-1)[0])
    s = -0.5 / float(np.exp(sigma)) ** 2   # coefficient on squared distances
    a = -2.0 * s                           # ACT scale

    emb = np.asarray(embeddings, dtype=np.float32)
    in_maps = build_in_maps(emb, s)
    nc = _build_bass(a)
    trace = bool(int(os.environ.get("KVV_TRACE", "0")))
    res = run_bass_kernel_spmd(nc, in_maps, core_ids=list(range(N_CORES)), trace=trace)
    if trace and res.exec_time_ns is not None:
        print(f"HW exec time: {res.exec_time_ns} ns")
        if res.mean_exec_time_ns is not None:
            print(f"HW exec time (mean across traced cores): {res.mean_exec_time_ns:.0f} ns")

    out = np.empty((B, N, N), dtype=np.float32)
    for core in range(N_CORES):
        b, half = divmod(core, 2)
        out[b, half * ROWS:(half + 1) * ROWS, :] = res.results[core]["out"]
    return out
